# revision 29
# baseline (speedup 1.0000x reference)
"""Multi-head self-attention (B=4, S=2048, E=1024, H=16) on 8 TRN2 NeuronCores.

Sharding: 8 cores = 4 batches x 2 sequence halves. Core c handles batch b=c//2,
query rows [h*1024, (h+1)*1024) with h=c%2. Each core computes Q/K/V for its own
row shard, the K/V shards are exchanged via one pairwise (2-rank) AllGather per
batch (bf16), and each core then runs full attention for its 16 heads over the
keys of its batch, followed by the full output projection for its rows. Host
only shards inputs, transposes/casts, and concatenates the per-core outputs.

Mask compaction (exactness-preserving): masked keys receive an additive -1e6
before exp, which underflows to an attention weight of exactly 0.0 in f32, and
masked queries are zeroed by the output mask -- so both are dropped on the
host. Each core keeps only its valid rows (compacted, zero-padded to NR, a
multiple of 128, uniform across cores); the kernel is compiled for that NR and
cached. Padded keys get -1e6 in negmask (weight 0), padded queries get 0 in
outmask, so results equal the full computation bit-for-bit up to bf16 noise.

Math notes (exactness-preserving rewrites):
- K bias dropped: adds a per-query constant to every score -> softmax invariant.
- V bias folded into the output-projection bias: bo_eff = WO @ bV + bO.
- 1/sqrt(D) and the additive key mask (-1e6 on masked keys) are fused into the
  exp activation: p = Exp(score/8 + negmask[key]).
- No max-subtraction in softmax: scores are O(1) here, exp cannot overflow.
- Softmax normalizer l rides as a ones-column in the V-hat stationary tiles;
  normalization is applied to the attention output (commutes with per-query
  scaling), via a K=1 ones-matmul that broadcasts 1/l across partitions.
"""

import sys
import os

if "/opt/trn_rl_repo" not in sys.path:
    sys.path.insert(0, "/opt/trn_rl_repo")

import numpy as np
import ml_dtypes

import concourse.bass as bass
import concourse.mybir as mybir
from concourse import bacc
from concourse.tile import TileContext
from concourse.bass_utils import run_bass_kernel_spmd

BF16 = mybir.dt.bfloat16
F32 = mybir.dt.float32

B, S, E, H = 4, 2048, 1024, 16
D = E // H          # 64
N_CORES = 8
ROWS = S // 2       # 1024 query rows per core before compaction
KT = E // 128       # 8 contraction tiles
JT = E // 128       # 8 output-feature tiles
ET = E // 128       # 8 e-tiles (head pairs)
FC = E // 512       # 2 feature chunks of 512
SCALE = 1.0 / 8.0   # 1/sqrt(D)

_prog_cache = {}


def _qchunks(nr):
    """Split nr rows into chunks of <=512 starting at multiples of 512, so a
    matmul output window never crosses a PSUM bank boundary (512 f32 = 2KB)."""
    out = []
    for pos in range(0, nr, 512):
        out.append((pos, min(512, nr - pos)))
    return out


def _build_program(nr, sim=False):
    """nr = padded valid rows per core (multiple of 128, <= 1024). sim=True
    builds a single-core variant for TimelineSim: the AllGathers are dropped
    and ag_k/ag_v become plain internal DRAM tensors (timing-only)."""
    njc2 = nr // 128          # own key chunks
    njc = 2 * njc2            # total key chunks (own + partner)
    nvt2 = nr // 128          # own v key-tiles
    nit = nr // 128           # query row-tiles
    qch = _qchunks(nr)        # query/key row chunks of <=512
    nic = len(qch)

    nc = bacc.Bacc("TRN2", target_bir_lowering=False, debug=False, num_devices=N_CORES)

    xT = nc.dram_tensor("xT", [E, nr], BF16, kind="ExternalInput").ap()
    wq = nc.dram_tensor("wq", [E, E], BF16, kind="ExternalInput").ap()
    wk = nc.dram_tensor("wk", [E, E], BF16, kind="ExternalInput").ap()
    wv = nc.dram_tensor("wv", [E, E], BF16, kind="ExternalInput").ap()
    wo = nc.dram_tensor("wo", [E, E], BF16, kind="ExternalInput").ap()
    bq = nc.dram_tensor("bq", [128, JT], F32, kind="ExternalInput").ap()
    negmask = nc.dram_tensor("negmask", [128, njc], F32, kind="ExternalInput").ap()
    outmask = nc.dram_tensor("outmask", [128, nit], F32, kind="ExternalInput").ap()
    bo_eff = nc.dram_tensor("bo_eff", [1, E], BF16, kind="ExternalInput").ap()
    pair_bk = nc.dram_tensor("pair_bk", [1, 1], mybir.dt.uint32, kind="ExternalInput").ap()
    pair_bv = nc.dram_tensor("pair_bv", [1, 1], mybir.dt.uint32, kind="ExternalInput").ap()
    out = nc.dram_tensor("out", [nr, E], BF16, kind="ExternalOutput").ap()

    with TileContext(nc) as tc:
        with (
            tc.tile_pool(name="persist", bufs=1) as persist,
            tc.tile_pool(name="dram", bufs=1, space="DRAM") as dram,
        ):
            # ---- persistent small tensors ----
            bq_t = persist.tile([128, JT], F32)
            nc.sync.dma_start(out=bq_t, in_=bq[:, :])
            nm_t = persist.tile([128, njc], F32)
            nc.sync.dma_start(out=nm_t, in_=negmask[:, :])
            om_t = persist.tile([128, nit], F32)
            nc.sync.dma_start(out=om_t, in_=outmask[:, :])
            ones_t = persist.tile([1, 128], BF16)
            nc.vector.memset(ones_t, 1.0)
            # ---- persistent big tensors ----
            ao_sb = [persist.tile([128, nr], BF16, name=f"ao{t}") for t in range(ET)]
            qT_sb = [persist.tile([128, nr], BF16, name=f"qT{j}") for j in range(JT)]
            # per-(pair,head) softmax denominators: head hh at partition hh*64
            la_sb = [persist.tile([D + 1, nr], BF16, name=f"la{t}") for t in range(ET)]

            # ---- bounce buffers for the pairwise K/V exchange ----
            # 2-rank AllGather only supports Local output; each core DMA-reads
            # the partner half into SBUF afterwards.
            bounce_k = dram.tile([E, nr], BF16)    # own K^T shard (feature-major)
            bounce_v = dram.tile([nr, E], BF16)    # own V shard (row-major)
            ag_k1 = dram.tile([E, nr], BF16, addr_space="Local")
            ag_k2 = dram.tile([E, nr], BF16, addr_space="Local")
            ag_v = dram.tile([2 * nr, E], BF16, addr_space="Local")

            # Key order on this core: [own nr | partner nr] (contraction over
            # keys is permutation-invariant; host builds negmask to match).
            # Attention runs in two sweeps: sweep A = all pairs x own keys
            # (starts as soon as K(0)/Q(0) exist; K/V/Q production interleaves
            # into the early pairs), sweep B = all pairs x partner keys (the
            # AllGathers complete behind sweep A's work).
            with (
                tc.tile_pool(name="p_xq", bufs=1) as p_xq,    # x + WQ
                tc.tile_pool(name="p_kst", bufs=8) as p_kst,  # K^T slots (own/partner share)
                tc.tile_pool(name="p_vh", bufs=1) as p_vh,    # all V-hat tiles
                tc.tile_pool(name="p_w", bufs=1) as p_w,      # WK + WV
                tc.tile_pool(name="p2s", bufs=3) as p2s,      # transients
                tc.tile_pool(name="psA", bufs=1, space="PSUM") as psA,
            ):
                xt = [p_xq.tile([128, nr], BF16, name=f"xt{k}") for k in range(KT)]
                wo_sb = [p_xq.tile([128, E], BF16, name=f"wo{k}") for k in range(KT)]
                wq_sb = [p_xq.tile([128, E], BF16, name=f"wq{k}") for k in range(KT)]
                kstage = [p_kst.tile([128, nr], BF16, name=f"kst{j}", tag="kst") for j in range(JT)]
                vhat = [p_vh.tile([128, H, D + 1], BF16, name=f"vh{v}") for v in range(2 * nvt2)]
                wk_sb = [p_w.tile([128, E], BF16, name=f"wk{k}") for k in range(KT)]
                wv_sb = [p_w.tile([128, E], BF16, name=f"wv{k}") for k in range(KT)]
                kpart = [p_kst.tile([128, nr], BF16, name=f"kp{j}", tag="kst") for j in range(JT)]

                # load order follows first use: x+WK, WQ, WV
                for k in range(KT):
                    nc.sync.dma_start(out=xt[k], in_=xT[k * 128:(k + 1) * 128, :])
                    nc.sync.dma_start(
                        out=wk_sb[k][:, 0:256], in_=wk[k * 128:(k + 1) * 128, 0:256]
                    )
                for k in range(KT):
                    nc.sync.dma_start(
                        out=wk_sb[k][:, 256:E], in_=wk[k * 128:(k + 1) * 128, 256:E]
                    )
                for k in range(KT):
                    nc.sync.dma_start(out=wq_sb[k], in_=wq[k * 128:(k + 1) * 128, :])
                for k in range(KT):
                    nc.sync.dma_start(out=wv_sb[k], in_=wv[k * 128:(k + 1) * 128, :])

                # "s" slots host transient accumulations (K/V/Q projections,
                # score tiles, normalization); "av" slots the attn@v chains.
                def s_tile(shape):
                    return psA.tile(shape, F32, name="ps_s", tag="s", bufs=2)

                def emit_k(j):
                    for s0, sz in qch:
                        sl = slice(s0, s0 + sz)
                        ps_k = s_tile([128, sz])
                        for k in range(KT):
                            nc.tensor.matmul(
                                ps_k, wk_sb[k][:, j * 128:(j + 1) * 128], xt[k][:, sl],
                                start=(k == 0), stop=(k == KT - 1),
                            )
                        nc.vector.tensor_copy(kstage[j][:, sl], ps_k)
                    nc.sync.dma_start(out=bounce_k[j * 128:(j + 1) * 128, :], in_=kstage[j])

                def emit_q(j):
                    for s0, sz in qch:
                        sl = slice(s0, s0 + sz)
                        ps_q = s_tile([128, sz])
                        for k in range(KT):
                            nc.tensor.matmul(
                                ps_q, wq_sb[k][:, j * 128:(j + 1) * 128], xt[k][:, sl],
                                start=(k == 0), stop=(k == KT - 1),
                            )
                        nc.vector.tensor_scalar_add(
                            qT_sb[j][:, sl], ps_q, bq_t[:, j:j + 1]
                        )

                def emit_v(v):
                    # V row-tile v (own keys v*128..): psum -> vhat directly
                    for fc in range(FC):
                        sl = slice(fc * 512, (fc + 1) * 512)
                        ps_v = s_tile([128, 512])
                        for k in range(KT):
                            nc.tensor.matmul(
                                ps_v, xt[k][:, v * 128:(v + 1) * 128], wv_sb[k][:, sl],
                                start=(k == 0), stop=(k == KT - 1),
                            )
                        nc.vector.tensor_copy(
                            vhat[v][:, 8 * fc:8 * (fc + 1), 0:D],
                            ps_v.rearrange("p (h d) -> p h d", h=8),
                        )
                    nc.vector.memset(vhat[v][:, :, D:D + 1], 1.0)
                    nc.sync.dma_start(
                        out=bounce_v[v * 128:(v + 1) * 128, :],
                        in_=vhat[v][:, :, 0:D],
                    )

                def emit_partner_v(v):
                    vtmp = p2s.tile([128, E], BF16, name="vtmp", tag="vtmp", bufs=2)
                    nc.sync.dma_start(
                        out=vtmp, in_=ag_v[bass.ds(base_v + v * 128, 128), :]
                    )
                    nc.vector.tensor_copy(
                        vhat[nvt2 + v][:, :, 0:D],
                        vtmp.rearrange("p (h d) -> p h d", h=H),
                    )
                    nc.vector.memset(vhat[nvt2 + v][:, :, D:D + 1], 1.0)

                def emit_norm(j):
                    # normalization of pair j: 1/l broadcast across partitions
                    # via a K=1 ones-matmul, then scale ao_sb in place.
                    for hh in range(2):
                        for s0, sz in qch:
                            sl = slice(s0, s0 + sz)
                            r32 = p2s.tile([1, sz], F32, name="r32", tag="r32", bufs=2)
                            nc.vector.reciprocal(r32, la_sb[j][hh * D:hh * D + 1, sl])
                            rbf = p2s.tile([1, sz], BF16, name="rbf", tag="rbf", bufs=2)
                            nc.vector.tensor_copy(rbf, r32)
                            ps_b = s_tile([D, sz])
                            nc.tensor.matmul(
                                ps_b, ones_t[:, 0:D], rbf, start=True, stop=True
                            )
                            nc.vector.tensor_mul(
                                ao_sb[j][hh * D:(hh + 1) * D, sl],
                                ao_sb[j][hh * D:(hh + 1) * D, sl],
                                ps_b,
                            )

                def emit_scores_exp(t, jc, kt_src, kcol):
                    phs = []
                    for hh in range(2):
                        prows = slice(hh * D, (hh + 1) * D)
                        ps_s = s_tile([128, nr])
                        for s0, sz in qch:
                            nc.tensor.matmul(
                                ps_s[:, s0:s0 + sz],
                                kt_src[prows, kcol:kcol + 128],
                                qT_sb[t][prows, s0:s0 + sz],
                                start=True, stop=True,
                                tile_position=(hh * D, 0),
                            )
                        ph = p2s.tile([128, nr], BF16, name="ph", tag="ph", bufs=6)
                        nc.scalar.activation(
                            ph, ps_s, mybir.ActivationFunctionType.Exp,
                            bias=nm_t[:, jc:jc + 1], scale=SCALE,
                        )
                        phs.append(ph)
                    return phs

                def emit_av(t, jc, phs, ps_av, first, last):
                    for hh in range(2):
                        h = 2 * t + hh
                        for ci, (s0, sz) in enumerate(qch):
                            nc.tensor.matmul(
                                ps_av[hh, ci],
                                vhat[jc][:, h, :],
                                phs[hh][:, s0:s0 + sz],
                                start=first, stop=last,
                            )

                emit_k(0)
                emit_q(0)
                bk_reg = nc.sync.alloc_register("bk_reg")
                nc.sync.reg_load(bk_reg, pair_bk[0:1, 0:1])
                base_k = nc.sync.snap(bk_reg, donate=True, min_val=0, max_val=E // 2)
                bv_reg = nc.sync.alloc_register("bv_reg")
                nc.sync.reg_load(bv_reg, pair_bv[0:1, 0:1])
                base_v = nc.sync.snap(bv_reg, donate=True, min_val=0, max_val=nr)

                # K(j>=1), Q(j>=1), V(all) interleave into sweep A's early
                # pairs: (pair, jc) -> list of emissions. Partner K is the
                # first thing sweep B consumes, so K emissions go two per
                # pair into pairs 0-2 and AG_K fires at pair 2, BEFORE AG_V
                # on the serial gpsimd queue (V partners are consumed later
                # than K partners); V tiles ride pair 0 one slot ahead of
                # their own-key av consumers; Q(t+2) two pairs ahead of use.
                last = njc2 - 1
                pre = {}
                pre[(0, min(1, last))] = [("k", 1)]
                pre.setdefault((0, min(2, last)), []).append(("q", 1))
                for j in range(2, JT):
                    slot = ((j - 2) // 2, min(2 + 2 * ((j - 2) % 2), last))
                    pre.setdefault(slot, []).append(("k", j))
                pre.setdefault((0, last), []).append(("agk", 0))
                pre.setdefault((2, last), []).append(("agk2", 0))
                pre.setdefault((3, min(1, last)), []).append(("agv", 0))
                for j in range(2, JT):
                    pre.setdefault((j - 2, min(6, last)), []).append(("q", j))
                for v in range(nvt2):
                    pre.setdefault((0, min(v + 1, last)), []).append(("v", v))

                LAG = 2
                # The pend queue carries ACROSS pair boundaries: while pair
                # t's last attn@v chains drain (waiting on their exps), pair
                # t+1's score matmuls already feed the PE, so the PE never
                # idles on the Activation engine at a boundary. ps_av tiles
                # are allocated lazily at a pair's first popped av (after the
                # previous pair's stash, so the 2*nic-slot rotation is safe),
                # and each pair is finalized (stash / accumulate) right after
                # its last av pops.
                av_state = {}
                pend = []

                def finalize(sw, t):
                    ps_av = av_state.pop((sw, t))
                    for hh in range(2):
                        for ci, (s0, sz) in enumerate(qch):
                            sl = slice(s0, s0 + sz)
                            av = ps_av[hh, ci]
                            if sw == 0:
                                # stash unnormalized partials + raw l
                                nc.vector.tensor_copy(
                                    la_sb[t][hh * D:hh * D + 1, sl], av[D:D + 1, :]
                                )
                                nc.vector.tensor_copy(
                                    ao_sb[t][hh * D:(hh + 1) * D, sl], av[0:D, :]
                                )
                            else:
                                # accumulate into partials; l += lB
                                nc.vector.tensor_add(
                                    la_sb[t][hh * D:hh * D + 1, sl],
                                    la_sb[t][hh * D:hh * D + 1, sl],
                                    av[D:D + 1, :],
                                )
                                nc.vector.tensor_add(
                                    ao_sb[t][hh * D:(hh + 1) * D, sl],
                                    ao_sb[t][hh * D:(hh + 1) * D, sl],
                                    av[0:D, :],
                                )

                def pop_one():
                    sw, t, j0, phs0, first, lst = pend.pop(0)
                    if (sw, t) not in av_state:
                        av_state[(sw, t)] = {
                            (hh, ci): psA.tile(
                                [D + 1, sz], F32, name="ps_av", tag="av",
                                bufs=2 * nic,
                            )
                            for hh in range(2)
                            for ci, (s0, sz) in enumerate(qch)
                        }
                    emit_av(t, j0, phs0, av_state[(sw, t)], first, lst)
                    if lst:
                        finalize(sw, t)

                # =============== sweep A: own keys ===============
                for t in range(ET):
                    for jc in range(njc2):
                        for kind, idx in pre.get((t, jc), ()):
                            if kind == "k":
                                emit_k(idx)
                            elif kind == "q":
                                emit_q(idx)
                            elif kind == "v":
                                emit_v(idx)
                            elif kind == "agk":
                                if not sim:
                                    nc.gpsimd.collective_compute(
                                        "AllGather", mybir.AluOpType.bypass,
                                        ins=[bounce_k[0:E // 2, :]],
                                        outs=[ag_k1[:, :]],
                                        replica_groups=[[2 * g, 2 * g + 1]
                                                        for g in range(N_CORES // 2)],
                                    )
                            elif kind == "agk2":
                                if not sim:
                                    nc.gpsimd.collective_compute(
                                        "AllGather", mybir.AluOpType.bypass,
                                        ins=[bounce_k[E // 2:E, :]],
                                        outs=[ag_k2[:, :]],
                                        replica_groups=[[2 * g, 2 * g + 1]
                                                        for g in range(N_CORES // 2)],
                                    )
                                for j in range(JT // 2):
                                    nc.sync.dma_start(
                                        out=kpart[j],
                                        in_=ag_k1[bass.ds(base_k + j * 128, 128), :],
                                    )
                                for j in range(JT // 2):
                                    nc.sync.dma_start(
                                        out=kpart[JT // 2 + j],
                                        in_=ag_k2[bass.ds(base_k + j * 128, 128), :],
                                    )
                            else:  # agv
                                if not sim:
                                    nc.gpsimd.collective_compute(
                                        "AllGather", mybir.AluOpType.bypass,
                                        ins=[bounce_v[:, :]], outs=[ag_v[:, :]],
                                        replica_groups=[[2 * g, 2 * g + 1]
                                                        for g in range(N_CORES // 2)],
                                    )
                                for v in range(nvt2):
                                    emit_partner_v(v)
                        pend.append((0, t, jc,
                                     emit_scores_exp(t, jc, kstage[t], jc * 128),
                                     jc == 0, jc == njc2 - 1))
                        while len(pend) > LAG:
                            pop_one()

                # =============== sweep B: partner keys ===============
                for t in range(ET):
                    for jc in range(njc2, njc):
                        pend.append((1, t, jc,
                                     emit_scores_exp(t, jc, kpart[t], (jc - njc2) * 128),
                                     jc == njc2, jc == njc - 1))
                        while len(pend) > LAG:
                            pop_one()
                        if jc == njc2 + min(3, njc2 - 1) and t > 0:
                            emit_norm(t - 1)  # previous pair: off the boundary
                while pend:
                    pop_one()
                emit_norm(ET - 1)

                # WO weights (loaded during attention), then the output
                # projection in the same pool/tag space -- no pool barrier.
                for k in range(KT):
                    nc.sync.dma_start(out=wo_sb[k], in_=wo[k * 128:(k + 1) * 128, :])

                bo_t = p2s.tile([1, E], BF16, name="bo_t", tag="bo", bufs=1)
                nc.sync.dma_start(out=bo_t, in_=bo_eff[:, :])

                for it in range(nit):
                    for fc in range(FC):
                        sl = slice(fc * 512, (fc + 1) * 512)
                        ps_o = psA.tile([128, 512], F32, name="ps_o", tag="av", bufs=2 * nic)
                        for k in range(KT):
                            nc.tensor.matmul(
                                ps_o,
                                ao_sb[k][:, it * 128:(it + 1) * 128],
                                wo_sb[k][:, sl],
                                start=(k == 0), stop=False,
                            )
                        nc.tensor.matmul(
                            ps_o, ones_t[:, 0:128], bo_t[:, sl],
                            start=False, stop=True,
                        )
                        o_sb = p2s.tile([128, 512], BF16, name="o_sb", tag="o_sb", bufs=3)
                        nc.scalar.activation(
                            o_sb, ps_o, mybir.ActivationFunctionType.Abs,
                            scale=om_t[:, it:it + 1],
                        )
                        nc.sync.dma_start(
                            out=out[it * 128:(it + 1) * 128, sl], in_=o_sb
                        )
    nc.compile()
    return nc


def _make_executor(nr):
    """Build the Bass program once and wrap it in a cached AOT-compiled
    shard_map. bass_exec declares a JAX effect, which forces the slow
    token-threaded Python dispatch path (~0.5-0.8 ms per call);
    fast_dispatch_compile suppresses the effect so calls take the C++ fast
    dispatch path."""
    import jax
    from jax.experimental.shard_map import shard_map
    from jax.sharding import Mesh, PartitionSpec, NamedSharding
    from concourse.bass2jax import (
        _bass_exec_p,
        install_neuronx_cc_hook,
        partition_id_tensor,
    )

    nc = _build_program(nr)
    install_neuronx_cc_hook()
    assert nc.dbg_addr is None
    partition_name = nc.partition_id_tensor.name if nc.partition_id_tensor else None

    in_names, out_names, out_avals, zero_outs = [], [], [], []
    for alloc in nc.m.functions[0].allocations:
        if not isinstance(alloc, mybir.MemoryLocationSet):
            continue
        name = alloc.memorylocations[0].name
        if alloc.kind == "ExternalInput":
            if name != partition_name:
                in_names.append(name)
        elif alloc.kind == "ExternalOutput":
            shape = tuple(alloc.tensor_shape)
            dtype = mybir.dt.np(alloc.dtype)
            out_names.append(name)
            out_avals.append(jax.core.ShapedArray(shape, dtype))
            zero_outs.append(np.zeros(shape, dtype))
    n_params = len(in_names)
    n_outs = len(out_avals)
    all_names = in_names + out_names
    if partition_name is not None:
        all_names = all_names + [partition_name]
    donate = tuple(range(n_params, n_params + n_outs))

    def _body(*args):
        operands = list(args)
        if partition_name is not None:
            operands.append(partition_id_tensor())
        outs = _bass_exec_p.bind(
            *operands,
            out_avals=tuple(out_avals),
            in_names=tuple(all_names),
            out_names=tuple(out_names),
            lowering_input_output_aliases=(),
            sim_require_finite=True,
            sim_require_nnan=True,
            nc=nc,
        )
        return tuple(outs)

    devices = jax.devices()[:N_CORES]
    mesh = Mesh(np.asarray(devices), ("core",))
    in_specs = (PartitionSpec("core"),) * (n_params + n_outs)
    out_specs = (PartitionSpec("core"),) * n_outs
    sharding = NamedSharding(mesh, PartitionSpec("core"))

    def _make_jit():
        return jax.jit(
            shard_map(_body, mesh=mesh, in_specs=in_specs, out_specs=out_specs,
                      check_rep=False),
            donate_argnums=donate,
            keep_unused=True,
        )

    try:
        from concourse.bass2jax import fast_dispatch_compile

        arg_shapes = []
        for alloc in nc.m.functions[0].allocations:
            if not isinstance(alloc, mybir.MemoryLocationSet):
                continue
            name = alloc.memorylocations[0].name
            if (alloc.kind == "ExternalInput" and name in in_names) or (
                alloc.kind == "ExternalOutput" and name in out_names
            ):
                shape = tuple(alloc.tensor_shape)
                dtype = mybir.dt.np(alloc.dtype)
                arg_shapes.append(
                    (name, jax.ShapeDtypeStruct(
                        (N_CORES * shape[0], *shape[1:]), dtype, sharding=sharding))
                )
        order = {n: i for i, n in enumerate(in_names + out_names)}
        args = [s for _, s in sorted(arg_shapes, key=lambda t: order[t[0]])]
        sharded = fast_dispatch_compile(
            lambda: _make_jit().lower(*args).compile()
        )
    except Exception:
        sharded = _make_jit()
    return {
        "jit": sharded, "in_names": in_names, "out_names": out_names,
        "out_avals": out_avals, "zero_outs": zero_outs, "sharding": sharding,
        "jax": jax,
    }


def get_executor(nr=None):
    if nr is None:
        nr = _prog_cache.get("nr", ROWS)
    key = ("ex", nr)
    if key not in _prog_cache:
        _prog_cache[key] = _make_executor(nr)
    return _prog_cache[key]


def run_spmd(in_maps, nr):
    """Execute on 8 cores; returns list of per-core output dicts.

    The axon tunnel occasionally drops the worker connection right after a
    heavy prior process ("notify failed ... hung up"); a fresh backend a
    minute later recovers. Retry by clearing jax backends and rebuilding the
    executor."""
    import time as _time

    last_err = None
    for attempt in range(3):
        try:
            ex = get_executor(nr)
            jax = ex["jax"]
            concat_in = [
                np.concatenate([np.asarray(m[name]) for m in in_maps], axis=0)
                for name in ex["in_names"]
            ]
            concat_zeros = [
                np.zeros((N_CORES * z.shape[0], *z.shape[1:]), z.dtype)
                for z in ex["zero_outs"]
            ]
            out_arrs = ex["jit"](*concat_in, *concat_zeros)
            out_arrs = [np.asarray(a) for a in out_arrs]
            return [
                {
                    name: out_arrs[i].reshape(N_CORES, *ex["out_avals"][i].shape)[c]
                    for i, name in enumerate(ex["out_names"])
                }
                for c in range(N_CORES)
            ]
        except Exception as e:  # transient tunnel failures
            last_err = e
            for key in [k for k in _prog_cache if isinstance(k, tuple)]:
                del _prog_cache[key]
            _time.sleep(45)
            try:
                import jax as _jax
                _jax.clear_caches()
                from jax._src import api as _japi
                _japi.clear_backends()
            except Exception:
                pass
    raise last_err


def _compaction(mask):
    """Per-core valid row indices + the uniform padded row count NR."""
    idxs = []
    for c in range(N_CORES):
        b, h = divmod(c, 2)
        own = mask[b, h * ROWS:(h + 1) * ROWS]
        idxs.append(np.nonzero(own)[0])
    vmax = max(len(i) for i in idxs)
    nr = max(128, -(-vmax // 128) * 128)
    return idxs, nr


def build_in_maps(x, mask, WQ_w, WQ_b, WK_w, WK_b, WV_w, WV_b, WO_w, WO_b):
    x = np.asarray(x, dtype=np.float32)
    mask = np.asarray(mask).astype(bool)
    WQ_w = np.asarray(WQ_w, dtype=np.float32)
    WQ_b = np.asarray(WQ_b, dtype=np.float32)
    WK_w = np.asarray(WK_w, dtype=np.float32)
    WV_w = np.asarray(WV_w, dtype=np.float32)
    WV_b = np.asarray(WV_b, dtype=np.float32)
    WO_w = np.asarray(WO_w, dtype=np.float32)
    WO_b = np.asarray(WO_b, dtype=np.float32)

    idxs, nr = _compaction(mask)
    _prog_cache["nr"] = nr
    njc2 = nr // 128
    nit = nr // 128

    wq_t = np.ascontiguousarray(WQ_w.T).astype(ml_dtypes.bfloat16)
    wk_t = np.ascontiguousarray(WK_w.T).astype(ml_dtypes.bfloat16)
    wv_t = np.ascontiguousarray(WV_w.T).astype(ml_dtypes.bfloat16)
    wo_t = np.ascontiguousarray(WO_w.T).astype(ml_dtypes.bfloat16)
    bq_t = np.ascontiguousarray(WQ_b.reshape(JT, 128).T)  # [128, JT] f32
    bo_eff = (WO_w @ WV_b + WO_b).astype(ml_dtypes.bfloat16).reshape(1, E)

    in_maps = []
    for c in range(N_CORES):
        b, h = divmod(c, 2)
        own_idx = idxs[c]
        part_idx = idxs[c ^ 1]
        nv, np_ = len(own_idx), len(part_idx)
        x_sh = np.zeros((nr, E), np.float32)
        x_sh[:nv] = x[b, h * ROWS + own_idx, :]
        xT_sh = np.ascontiguousarray(x_sh.T).astype(ml_dtypes.bfloat16)
        # key order on this core: [own compacted | partner compacted]
        negm = np.full(2 * nr, -1e6, np.float32)
        negm[:nv] = 0.0
        negm[nr:nr + np_] = 0.0
        nm_t = np.ascontiguousarray(negm.reshape(2 * njc2, 128).T)   # [128, njc]
        om = np.zeros(nr, np.float32)
        om[:nv] = 1.0
        om_t = np.ascontiguousarray(om.reshape(nit, 128).T)          # [128, nit]
        in_maps.append({
            "xT": xT_sh, "wq": wq_t, "wk": wk_t, "wv": wv_t, "wo": wo_t,
            "bq": bq_t, "negmask": nm_t, "outmask": om_t, "bo_eff": bo_eff,
            "pair_bk": np.array([[(1 - h) * (E // 2)]], dtype=np.uint32),
            "pair_bv": np.array([[(1 - h) * nr]], dtype=np.uint32),
        })
    return in_maps


def kernel(x, mask, WQ_w, WQ_b, WK_w, WK_b, WV_w, WV_b, WO_w, WO_b):
    mask = np.asarray(mask).astype(bool)
    in_maps = build_in_maps(x, mask, WQ_w, WQ_b, WK_w, WK_b, WV_w, WV_b, WO_w, WO_b)
    idxs, nr = _compaction(mask)
    results = run_spmd(in_maps, nr)
    out = np.zeros((B, S, E), dtype=np.float32)
    for c in range(N_CORES):
        b, h = divmod(c, 2)
        nv = len(idxs[c])
        out[b, h * ROWS + idxs[c], :] = results[c]["out"][:nv].astype(np.float32)
    return out


# revision 30
# speedup vs baseline: 1.0102x; 1.0102x over previous
"""Multi-head self-attention (B=4, S=2048, E=1024, H=16) on 8 TRN2 NeuronCores.

Sharding: 8 cores = 4 batches x 2 sequence halves. Core c handles batch b=c//2,
query rows [h*1024, (h+1)*1024) with h=c%2. Each core computes Q/K/V for its own
row shard, the K/V shards are exchanged via one pairwise (2-rank) AllGather per
batch (bf16), and each core then runs full attention for its 16 heads over the
keys of its batch, followed by the full output projection for its rows. Host
only shards inputs, transposes/casts, and concatenates the per-core outputs.

Mask compaction (exactness-preserving): masked keys receive an additive -1e6
before exp, which underflows to an attention weight of exactly 0.0 in f32, and
masked queries are zeroed by the output mask -- so both are dropped on the
host. Each core keeps only its valid rows (compacted, zero-padded to NR, a
multiple of 128, uniform across cores); the kernel is compiled for that NR and
cached. Padded keys get -1e6 in negmask (weight 0), padded queries get 0 in
outmask, so results equal the full computation bit-for-bit up to bf16 noise.

Math notes (exactness-preserving rewrites):
- K bias dropped: adds a per-query constant to every score -> softmax invariant.
- V bias folded into the output-projection bias: bo_eff = WO @ bV + bO.
- 1/sqrt(D) and the additive key mask (-1e6 on masked keys) are fused into the
  exp activation: p = Exp(score/8 + negmask[key]).
- No max-subtraction in softmax: scores are O(1) here, exp cannot overflow.
- Softmax normalizer l rides as a ones-column in the V-hat stationary tiles;
  normalization is applied to the attention output (commutes with per-query
  scaling), via a K=1 ones-matmul that broadcasts 1/l across partitions.
"""

import sys
import os

if "/opt/trn_rl_repo" not in sys.path:
    sys.path.insert(0, "/opt/trn_rl_repo")

import numpy as np
import ml_dtypes

import concourse.bass as bass
import concourse.mybir as mybir
from concourse import bacc
from concourse.tile import TileContext
from concourse.bass_utils import run_bass_kernel_spmd

BF16 = mybir.dt.bfloat16
F32 = mybir.dt.float32

B, S, E, H = 4, 2048, 1024, 16
D = E // H          # 64
N_CORES = 8
ROWS = S // 2       # 1024 query rows per core before compaction
KT = E // 128       # 8 contraction tiles
JT = E // 128       # 8 output-feature tiles
ET = E // 128       # 8 e-tiles (head pairs)
FC = E // 512       # 2 feature chunks of 512
SCALE = 1.0 / 8.0   # 1/sqrt(D)

_prog_cache = {}


def _qchunks(nr):
    """Split nr rows into chunks of <=512 starting at multiples of 512, so a
    matmul output window never crosses a PSUM bank boundary (512 f32 = 2KB)."""
    out = []
    for pos in range(0, nr, 512):
        out.append((pos, min(512, nr - pos)))
    return out


def _build_program(nr, sim=False):
    """nr = padded valid rows per core (multiple of 128, <= 1024). sim=True
    builds a single-core variant for TimelineSim: the AllGathers are dropped
    and ag_k/ag_v become plain internal DRAM tensors (timing-only)."""
    njc2 = nr // 128          # own key chunks
    njc = 2 * njc2            # total key chunks (own + partner)
    nvt2 = nr // 128          # own v key-tiles
    nit = nr // 128           # query row-tiles
    qch = _qchunks(nr)        # query/key row chunks of <=512
    nic = len(qch)

    nc = bacc.Bacc("TRN2", target_bir_lowering=False, debug=False, num_devices=N_CORES)

    xT = nc.dram_tensor("xT", [E, nr], BF16, kind="ExternalInput").ap()
    wq = nc.dram_tensor("wq", [E, E], BF16, kind="ExternalInput").ap()
    wk = nc.dram_tensor("wk", [E, E], BF16, kind="ExternalInput").ap()
    wv = nc.dram_tensor("wv", [E, E], BF16, kind="ExternalInput").ap()
    wo = nc.dram_tensor("wo", [E, E], BF16, kind="ExternalInput").ap()
    bq = nc.dram_tensor("bq", [128, JT], F32, kind="ExternalInput").ap()
    negmask = nc.dram_tensor("negmask", [128, njc], F32, kind="ExternalInput").ap()
    outmask = nc.dram_tensor("outmask", [128, nit], F32, kind="ExternalInput").ap()
    bo_eff = nc.dram_tensor("bo_eff", [1, E], BF16, kind="ExternalInput").ap()
    pair_bk = nc.dram_tensor("pair_bk", [1, 1], mybir.dt.uint32, kind="ExternalInput").ap()
    pair_bv = nc.dram_tensor("pair_bv", [1, 1], mybir.dt.uint32, kind="ExternalInput").ap()
    out = nc.dram_tensor("out", [nr, E], BF16, kind="ExternalOutput").ap()

    with TileContext(nc) as tc:
        with (
            tc.tile_pool(name="persist", bufs=1) as persist,
            tc.tile_pool(name="dram", bufs=1, space="DRAM") as dram,
        ):
            # ---- persistent small tensors ----
            bq_t = persist.tile([128, JT], F32)
            nc.sync.dma_start(out=bq_t, in_=bq[:, :])
            nm_t = persist.tile([128, njc], F32)
            nc.sync.dma_start(out=nm_t, in_=negmask[:, :])
            om_t = persist.tile([128, nit], F32)
            nc.sync.dma_start(out=om_t, in_=outmask[:, :])
            ones_t = persist.tile([1, 128], BF16)
            nc.vector.memset(ones_t, 1.0)
            # ---- persistent big tensors ----
            ao_sb = [persist.tile([128, nr], BF16, name=f"ao{t}") for t in range(ET)]
            qT_sb = [persist.tile([128, nr], BF16, name=f"qT{j}") for j in range(JT)]
            # per-(pair,head) softmax denominators: head hh at partition hh*64
            la_sb = [persist.tile([D + 1, nr], BF16, name=f"la{t}") for t in range(ET)]

            # ---- bounce buffers for the pairwise K/V exchange ----
            # 2-rank AllGather only supports Local output; each core DMA-reads
            # the partner half into SBUF afterwards.
            bounce_k = dram.tile([E, nr], BF16)    # own K^T shard (feature-major)
            bounce_v = dram.tile([nr, E], BF16)    # own V shard (row-major)
            ag_k1 = dram.tile([E, nr], BF16, addr_space="Local")
            ag_k2 = dram.tile([E, nr], BF16, addr_space="Local")
            ag_v = dram.tile([2 * nr, E], BF16, addr_space="Local")

            # Key order on this core: [own nr | partner nr] (contraction over
            # keys is permutation-invariant; host builds negmask to match).
            # Attention runs in two sweeps: sweep A = all pairs x own keys
            # (starts as soon as K(0)/Q(0) exist; K/V/Q production interleaves
            # into the early pairs), sweep B = all pairs x partner keys (the
            # AllGathers complete behind sweep A's work).
            with (
                tc.tile_pool(name="p_xq", bufs=1) as p_xq,    # x + WQ
                tc.tile_pool(name="p_kst", bufs=8) as p_kst,  # K^T slots (own/partner share)
                tc.tile_pool(name="p_vh", bufs=1) as p_vh,    # all V-hat tiles
                tc.tile_pool(name="p_w", bufs=1) as p_w,      # WK + WV
                tc.tile_pool(name="p2s", bufs=3) as p2s,      # transients
                tc.tile_pool(name="psA", bufs=1, space="PSUM") as psA,
            ):
                xt = [p_xq.tile([128, nr], BF16, name=f"xt{k}") for k in range(KT)]
                wo_sb = [p_xq.tile([128, E], BF16, name=f"wo{k}") for k in range(KT)]
                wq_sb = [p_xq.tile([128, E], BF16, name=f"wq{k}") for k in range(KT)]
                kstage = [p_kst.tile([128, nr], BF16, name=f"kst{j}", tag="kst") for j in range(JT)]
                vhat = [p_vh.tile([128, H, D + 1], BF16, name=f"vh{v}") for v in range(2 * nvt2)]
                wk_sb = [p_w.tile([128, E], BF16, name=f"wk{k}") for k in range(KT)]
                wv_sb = [p_w.tile([128, E], BF16, name=f"wv{k}") for k in range(KT)]
                kpart = [p_kst.tile([128, nr], BF16, name=f"kp{j}", tag="kst") for j in range(JT)]

                # load order follows first use: x+WK, WQ, WV
                for k in range(KT):
                    nc.sync.dma_start(out=xt[k], in_=xT[k * 128:(k + 1) * 128, :])
                    nc.sync.dma_start(
                        out=wk_sb[k][:, 0:256], in_=wk[k * 128:(k + 1) * 128, 0:256]
                    )
                for k in range(KT):
                    nc.sync.dma_start(
                        out=wk_sb[k][:, 256:E], in_=wk[k * 128:(k + 1) * 128, 256:E]
                    )
                for k in range(KT):
                    nc.sync.dma_start(out=wq_sb[k], in_=wq[k * 128:(k + 1) * 128, :])
                for k in range(KT):
                    nc.sync.dma_start(out=wv_sb[k], in_=wv[k * 128:(k + 1) * 128, :])

                # "s" slots host transient accumulations (K/V/Q projections,
                # score tiles, normalization); "av" slots the attn@v chains.
                def s_tile(shape):
                    return psA.tile(shape, F32, name="ps_s", tag="s", bufs=2)

                def emit_k(j):
                    for s0, sz in qch:
                        sl = slice(s0, s0 + sz)
                        ps_k = s_tile([128, sz])
                        for k in range(KT):
                            nc.tensor.matmul(
                                ps_k, wk_sb[k][:, j * 128:(j + 1) * 128], xt[k][:, sl],
                                start=(k == 0), stop=(k == KT - 1),
                            )
                        nc.vector.tensor_copy(kstage[j][:, sl], ps_k)
                    nc.sync.dma_start(out=bounce_k[j * 128:(j + 1) * 128, :], in_=kstage[j])

                def emit_q(j):
                    for s0, sz in qch:
                        sl = slice(s0, s0 + sz)
                        ps_q = s_tile([128, sz])
                        for k in range(KT):
                            nc.tensor.matmul(
                                ps_q, wq_sb[k][:, j * 128:(j + 1) * 128], xt[k][:, sl],
                                start=(k == 0), stop=(k == KT - 1),
                            )
                        nc.vector.tensor_scalar_add(
                            qT_sb[j][:, sl], ps_q, bq_t[:, j:j + 1]
                        )

                def emit_v(v):
                    # V row-tile v (own keys v*128..): psum -> vhat directly
                    for fc in range(FC):
                        sl = slice(fc * 512, (fc + 1) * 512)
                        ps_v = s_tile([128, 512])
                        for k in range(KT):
                            nc.tensor.matmul(
                                ps_v, xt[k][:, v * 128:(v + 1) * 128], wv_sb[k][:, sl],
                                start=(k == 0), stop=(k == KT - 1),
                            )
                        nc.vector.tensor_copy(
                            vhat[v][:, 8 * fc:8 * (fc + 1), 0:D],
                            ps_v.rearrange("p (h d) -> p h d", h=8),
                        )
                    nc.vector.memset(vhat[v][:, :, D:D + 1], 1.0)
                    nc.sync.dma_start(
                        out=bounce_v[v * 128:(v + 1) * 128, :],
                        in_=vhat[v][:, :, 0:D],
                    )

                def emit_partner_v(v):
                    vtmp = p2s.tile([128, E], BF16, name="vtmp", tag="vtmp", bufs=2)
                    nc.sync.dma_start(
                        out=vtmp, in_=ag_v[bass.ds(base_v + v * 128, 128), :]
                    )
                    nc.vector.tensor_copy(
                        vhat[nvt2 + v][:, :, 0:D],
                        vtmp.rearrange("p (h d) -> p h d", h=H),
                    )
                    nc.vector.memset(vhat[nvt2 + v][:, :, D:D + 1], 1.0)

                def emit_norm(j):
                    # normalization of pair j: 1/l broadcast across partitions
                    # via a K=1 ones-matmul, then scale ao_sb in place. DVE
                    # ops run full-width; only the PSUM-writing matmuls stay
                    # chunked (512-aligned windows, no bank crossing).
                    for hh in range(2):
                        r32 = p2s.tile([1, nr], F32, name="r32", tag="r32", bufs=2)
                        nc.vector.reciprocal(r32, la_sb[j][hh * D:hh * D + 1, :])
                        rbf = p2s.tile([1, nr], BF16, name="rbf", tag="rbf", bufs=2)
                        nc.vector.tensor_copy(rbf, r32)
                        ps_b = s_tile([D, nr])
                        for s0, sz in qch:
                            nc.tensor.matmul(
                                ps_b[:, s0:s0 + sz], ones_t[:, 0:D],
                                rbf[:, s0:s0 + sz], start=True, stop=True,
                            )
                        nc.vector.tensor_mul(
                            ao_sb[j][hh * D:(hh + 1) * D, :],
                            ao_sb[j][hh * D:(hh + 1) * D, :],
                            ps_b,
                        )

                def emit_scores_exp(t, jc, kt_src, kcol):
                    phs = []
                    for hh in range(2):
                        prows = slice(hh * D, (hh + 1) * D)
                        ps_s = s_tile([128, nr])
                        for s0, sz in qch:
                            nc.tensor.matmul(
                                ps_s[:, s0:s0 + sz],
                                kt_src[prows, kcol:kcol + 128],
                                qT_sb[t][prows, s0:s0 + sz],
                                start=True, stop=True,
                                tile_position=(hh * D, 0),
                            )
                        ph = p2s.tile([128, nr], BF16, name="ph", tag="ph", bufs=6)
                        nc.scalar.activation(
                            ph, ps_s, mybir.ActivationFunctionType.Exp,
                            bias=nm_t[:, jc:jc + 1], scale=SCALE,
                        )
                        phs.append(ph)
                    return phs

                def emit_av(t, jc, phs, ps_av, first, last):
                    for hh in range(2):
                        h = 2 * t + hh
                        for ci, (s0, sz) in enumerate(qch):
                            nc.tensor.matmul(
                                ps_av[hh, ci],
                                vhat[jc][:, h, :],
                                phs[hh][:, s0:s0 + sz],
                                start=first, stop=last,
                            )

                emit_k(0)
                emit_q(0)
                bk_reg = nc.sync.alloc_register("bk_reg")
                nc.sync.reg_load(bk_reg, pair_bk[0:1, 0:1])
                base_k = nc.sync.snap(bk_reg, donate=True, min_val=0, max_val=E // 2)
                bv_reg = nc.sync.alloc_register("bv_reg")
                nc.sync.reg_load(bv_reg, pair_bv[0:1, 0:1])
                base_v = nc.sync.snap(bv_reg, donate=True, min_val=0, max_val=nr)

                # K(j>=1), Q(j>=1), V(all) interleave into sweep A's early
                # pairs: (pair, jc) -> list of emissions. Partner K is the
                # first thing sweep B consumes, so K emissions go two per
                # pair into pairs 0-2 and AG_K fires at pair 2, BEFORE AG_V
                # on the serial gpsimd queue (V partners are consumed later
                # than K partners); V tiles ride pair 0 one slot ahead of
                # their own-key av consumers; Q(t+2) two pairs ahead of use.
                last = njc2 - 1
                pre = {}
                pre[(0, min(1, last))] = [("k", 1)]
                pre.setdefault((0, min(2, last)), []).append(("q", 1))
                for j in range(2, JT):
                    slot = ((j - 2) // 2, min(2 + 2 * ((j - 2) % 2), last))
                    pre.setdefault(slot, []).append(("k", j))
                pre.setdefault((0, last), []).append(("agk", 0))
                pre.setdefault((2, last), []).append(("agk2", 0))
                pre.setdefault((3, min(1, last)), []).append(("agv", 0))
                for j in range(2, JT):
                    pre.setdefault((j - 2, min(6, last)), []).append(("q", j))
                for v in range(nvt2):
                    pre.setdefault((0, min(v + 1, last)), []).append(("v", v))

                LAG = 2
                # The pend queue carries ACROSS pair boundaries: while pair
                # t's last attn@v chains drain (waiting on their exps), pair
                # t+1's score matmuls already feed the PE, so the PE never
                # idles on the Activation engine at a boundary. ps_av tiles
                # are allocated lazily at a pair's first popped av (after the
                # previous pair's stash, so the 2*nic-slot rotation is safe),
                # and each pair is finalized (stash / accumulate) right after
                # its last av pops.
                av_state = {}
                pend = []

                def finalize(sw, t):
                    ps_av = av_state.pop((sw, t))
                    for hh in range(2):
                        for ci, (s0, sz) in enumerate(qch):
                            sl = slice(s0, s0 + sz)
                            av = ps_av[hh, ci]
                            if sw == 0:
                                # stash unnormalized partials + raw l
                                nc.vector.tensor_copy(
                                    la_sb[t][hh * D:hh * D + 1, sl], av[D:D + 1, :]
                                )
                                nc.vector.tensor_copy(
                                    ao_sb[t][hh * D:(hh + 1) * D, sl], av[0:D, :]
                                )
                            else:
                                # accumulate into partials; l += lB
                                nc.vector.tensor_add(
                                    la_sb[t][hh * D:hh * D + 1, sl],
                                    la_sb[t][hh * D:hh * D + 1, sl],
                                    av[D:D + 1, :],
                                )
                                nc.vector.tensor_add(
                                    ao_sb[t][hh * D:(hh + 1) * D, sl],
                                    ao_sb[t][hh * D:(hh + 1) * D, sl],
                                    av[0:D, :],
                                )

                def pop_one():
                    sw, t, j0, phs0, first, lst = pend.pop(0)
                    if (sw, t) not in av_state:
                        av_state[(sw, t)] = {
                            (hh, ci): psA.tile(
                                [D + 1, sz], F32, name="ps_av", tag="av",
                                bufs=2 * nic,
                            )
                            for hh in range(2)
                            for ci, (s0, sz) in enumerate(qch)
                        }
                    emit_av(t, j0, phs0, av_state[(sw, t)], first, lst)
                    if lst:
                        finalize(sw, t)

                # =============== sweep A: own keys ===============
                for t in range(ET):
                    for jc in range(njc2):
                        for kind, idx in pre.get((t, jc), ()):
                            if kind == "k":
                                emit_k(idx)
                            elif kind == "q":
                                emit_q(idx)
                            elif kind == "v":
                                emit_v(idx)
                            elif kind == "agk":
                                if not sim:
                                    nc.gpsimd.collective_compute(
                                        "AllGather", mybir.AluOpType.bypass,
                                        ins=[bounce_k[0:E // 2, :]],
                                        outs=[ag_k1[:, :]],
                                        replica_groups=[[2 * g, 2 * g + 1]
                                                        for g in range(N_CORES // 2)],
                                    )
                            elif kind == "agk2":
                                if not sim:
                                    nc.gpsimd.collective_compute(
                                        "AllGather", mybir.AluOpType.bypass,
                                        ins=[bounce_k[E // 2:E, :]],
                                        outs=[ag_k2[:, :]],
                                        replica_groups=[[2 * g, 2 * g + 1]
                                                        for g in range(N_CORES // 2)],
                                    )
                                for j in range(JT // 2):
                                    nc.sync.dma_start(
                                        out=kpart[j],
                                        in_=ag_k1[bass.ds(base_k + j * 128, 128), :],
                                    )
                                for j in range(JT // 2):
                                    nc.sync.dma_start(
                                        out=kpart[JT // 2 + j],
                                        in_=ag_k2[bass.ds(base_k + j * 128, 128), :],
                                    )
                            else:  # agv
                                if not sim:
                                    nc.gpsimd.collective_compute(
                                        "AllGather", mybir.AluOpType.bypass,
                                        ins=[bounce_v[:, :]], outs=[ag_v[:, :]],
                                        replica_groups=[[2 * g, 2 * g + 1]
                                                        for g in range(N_CORES // 2)],
                                    )
                                for v in range(nvt2):
                                    emit_partner_v(v)
                        pend.append((0, t, jc,
                                     emit_scores_exp(t, jc, kstage[t], jc * 128),
                                     jc == 0, jc == njc2 - 1))
                        while len(pend) > LAG:
                            pop_one()

                # =============== sweep B: partner keys ===============
                for t in range(ET):
                    for jc in range(njc2, njc):
                        pend.append((1, t, jc,
                                     emit_scores_exp(t, jc, kpart[t], (jc - njc2) * 128),
                                     jc == njc2, jc == njc - 1))
                        while len(pend) > LAG:
                            pop_one()
                        if jc == njc2 + min(3, njc2 - 1) and t > 0:
                            emit_norm(t - 1)  # previous pair: off the boundary
                while pend:
                    pop_one()
                emit_norm(ET - 1)

                # WO weights (loaded during attention), then the output
                # projection in the same pool/tag space -- no pool barrier.
                for k in range(KT):
                    nc.sync.dma_start(out=wo_sb[k], in_=wo[k * 128:(k + 1) * 128, :])

                bo_t = p2s.tile([1, E], BF16, name="bo_t", tag="bo", bufs=1)
                nc.sync.dma_start(out=bo_t, in_=bo_eff[:, :])

                for it in range(nit):
                    for fc in range(FC):
                        sl = slice(fc * 512, (fc + 1) * 512)
                        ps_o = psA.tile([128, 512], F32, name="ps_o", tag="av", bufs=2 * nic)
                        for k in range(KT):
                            nc.tensor.matmul(
                                ps_o,
                                ao_sb[k][:, it * 128:(it + 1) * 128],
                                wo_sb[k][:, sl],
                                start=(k == 0), stop=False,
                            )
                        nc.tensor.matmul(
                            ps_o, ones_t[:, 0:128], bo_t[:, sl],
                            start=False, stop=True,
                        )
                        o_sb = p2s.tile([128, 512], BF16, name="o_sb", tag="o_sb", bufs=3)
                        nc.scalar.activation(
                            o_sb, ps_o, mybir.ActivationFunctionType.Abs,
                            scale=om_t[:, it:it + 1],
                        )
                        nc.sync.dma_start(
                            out=out[it * 128:(it + 1) * 128, sl], in_=o_sb
                        )
    nc.compile()
    return nc


def _make_executor(nr):
    """Build the Bass program once and wrap it in a cached AOT-compiled
    shard_map. bass_exec declares a JAX effect, which forces the slow
    token-threaded Python dispatch path (~0.5-0.8 ms per call);
    fast_dispatch_compile suppresses the effect so calls take the C++ fast
    dispatch path."""
    import jax
    from jax.experimental.shard_map import shard_map
    from jax.sharding import Mesh, PartitionSpec, NamedSharding
    from concourse.bass2jax import (
        _bass_exec_p,
        install_neuronx_cc_hook,
        partition_id_tensor,
    )

    nc = _build_program(nr)
    install_neuronx_cc_hook()
    assert nc.dbg_addr is None
    partition_name = nc.partition_id_tensor.name if nc.partition_id_tensor else None

    in_names, out_names, out_avals, zero_outs = [], [], [], []
    for alloc in nc.m.functions[0].allocations:
        if not isinstance(alloc, mybir.MemoryLocationSet):
            continue
        name = alloc.memorylocations[0].name
        if alloc.kind == "ExternalInput":
            if name != partition_name:
                in_names.append(name)
        elif alloc.kind == "ExternalOutput":
            shape = tuple(alloc.tensor_shape)
            dtype = mybir.dt.np(alloc.dtype)
            out_names.append(name)
            out_avals.append(jax.core.ShapedArray(shape, dtype))
            zero_outs.append(np.zeros(shape, dtype))
    n_params = len(in_names)
    n_outs = len(out_avals)
    all_names = in_names + out_names
    if partition_name is not None:
        all_names = all_names + [partition_name]
    donate = tuple(range(n_params, n_params + n_outs))

    def _body(*args):
        operands = list(args)
        if partition_name is not None:
            operands.append(partition_id_tensor())
        outs = _bass_exec_p.bind(
            *operands,
            out_avals=tuple(out_avals),
            in_names=tuple(all_names),
            out_names=tuple(out_names),
            lowering_input_output_aliases=(),
            sim_require_finite=True,
            sim_require_nnan=True,
            nc=nc,
        )
        return tuple(outs)

    devices = jax.devices()[:N_CORES]
    mesh = Mesh(np.asarray(devices), ("core",))
    in_specs = (PartitionSpec("core"),) * (n_params + n_outs)
    out_specs = (PartitionSpec("core"),) * n_outs
    sharding = NamedSharding(mesh, PartitionSpec("core"))

    def _make_jit():
        return jax.jit(
            shard_map(_body, mesh=mesh, in_specs=in_specs, out_specs=out_specs,
                      check_rep=False),
            donate_argnums=donate,
            keep_unused=True,
        )

    try:
        from concourse.bass2jax import fast_dispatch_compile

        arg_shapes = []
        for alloc in nc.m.functions[0].allocations:
            if not isinstance(alloc, mybir.MemoryLocationSet):
                continue
            name = alloc.memorylocations[0].name
            if (alloc.kind == "ExternalInput" and name in in_names) or (
                alloc.kind == "ExternalOutput" and name in out_names
            ):
                shape = tuple(alloc.tensor_shape)
                dtype = mybir.dt.np(alloc.dtype)
                arg_shapes.append(
                    (name, jax.ShapeDtypeStruct(
                        (N_CORES * shape[0], *shape[1:]), dtype, sharding=sharding))
                )
        order = {n: i for i, n in enumerate(in_names + out_names)}
        args = [s for _, s in sorted(arg_shapes, key=lambda t: order[t[0]])]
        sharded = fast_dispatch_compile(
            lambda: _make_jit().lower(*args).compile()
        )
    except Exception:
        sharded = _make_jit()
    return {
        "jit": sharded, "in_names": in_names, "out_names": out_names,
        "out_avals": out_avals, "zero_outs": zero_outs, "sharding": sharding,
        "jax": jax,
    }


def get_executor(nr=None):
    if nr is None:
        nr = _prog_cache.get("nr", ROWS)
    key = ("ex", nr)
    if key not in _prog_cache:
        _prog_cache[key] = _make_executor(nr)
    return _prog_cache[key]


def run_spmd(in_maps, nr):
    """Execute on 8 cores; returns list of per-core output dicts.

    The axon tunnel occasionally drops the worker connection right after a
    heavy prior process ("notify failed ... hung up"); a fresh backend a
    minute later recovers. Retry by clearing jax backends and rebuilding the
    executor."""
    import time as _time

    last_err = None
    for attempt in range(3):
        try:
            ex = get_executor(nr)
            jax = ex["jax"]
            concat_in = [
                np.concatenate([np.asarray(m[name]) for m in in_maps], axis=0)
                for name in ex["in_names"]
            ]
            concat_zeros = [
                np.zeros((N_CORES * z.shape[0], *z.shape[1:]), z.dtype)
                for z in ex["zero_outs"]
            ]
            out_arrs = ex["jit"](*concat_in, *concat_zeros)
            out_arrs = [np.asarray(a) for a in out_arrs]
            return [
                {
                    name: out_arrs[i].reshape(N_CORES, *ex["out_avals"][i].shape)[c]
                    for i, name in enumerate(ex["out_names"])
                }
                for c in range(N_CORES)
            ]
        except Exception as e:  # transient tunnel failures
            last_err = e
            for key in [k for k in _prog_cache if isinstance(k, tuple)]:
                del _prog_cache[key]
            _time.sleep(45)
            try:
                import jax as _jax
                _jax.clear_caches()
                from jax._src import api as _japi
                _japi.clear_backends()
            except Exception:
                pass
    raise last_err


def _compaction(mask):
    """Per-core valid row indices + the uniform padded row count NR."""
    idxs = []
    for c in range(N_CORES):
        b, h = divmod(c, 2)
        own = mask[b, h * ROWS:(h + 1) * ROWS]
        idxs.append(np.nonzero(own)[0])
    vmax = max(len(i) for i in idxs)
    nr = max(128, -(-vmax // 128) * 128)
    return idxs, nr


def build_in_maps(x, mask, WQ_w, WQ_b, WK_w, WK_b, WV_w, WV_b, WO_w, WO_b):
    x = np.asarray(x, dtype=np.float32)
    mask = np.asarray(mask).astype(bool)
    WQ_w = np.asarray(WQ_w, dtype=np.float32)
    WQ_b = np.asarray(WQ_b, dtype=np.float32)
    WK_w = np.asarray(WK_w, dtype=np.float32)
    WV_w = np.asarray(WV_w, dtype=np.float32)
    WV_b = np.asarray(WV_b, dtype=np.float32)
    WO_w = np.asarray(WO_w, dtype=np.float32)
    WO_b = np.asarray(WO_b, dtype=np.float32)

    idxs, nr = _compaction(mask)
    _prog_cache["nr"] = nr
    njc2 = nr // 128
    nit = nr // 128

    wq_t = np.ascontiguousarray(WQ_w.T).astype(ml_dtypes.bfloat16)
    wk_t = np.ascontiguousarray(WK_w.T).astype(ml_dtypes.bfloat16)
    wv_t = np.ascontiguousarray(WV_w.T).astype(ml_dtypes.bfloat16)
    wo_t = np.ascontiguousarray(WO_w.T).astype(ml_dtypes.bfloat16)
    bq_t = np.ascontiguousarray(WQ_b.reshape(JT, 128).T)  # [128, JT] f32
    bo_eff = (WO_w @ WV_b + WO_b).astype(ml_dtypes.bfloat16).reshape(1, E)

    in_maps = []
    for c in range(N_CORES):
        b, h = divmod(c, 2)
        own_idx = idxs[c]
        part_idx = idxs[c ^ 1]
        nv, np_ = len(own_idx), len(part_idx)
        x_sh = np.zeros((nr, E), np.float32)
        x_sh[:nv] = x[b, h * ROWS + own_idx, :]
        xT_sh = np.ascontiguousarray(x_sh.T).astype(ml_dtypes.bfloat16)
        # key order on this core: [own compacted | partner compacted]
        negm = np.full(2 * nr, -1e6, np.float32)
        negm[:nv] = 0.0
        negm[nr:nr + np_] = 0.0
        nm_t = np.ascontiguousarray(negm.reshape(2 * njc2, 128).T)   # [128, njc]
        om = np.zeros(nr, np.float32)
        om[:nv] = 1.0
        om_t = np.ascontiguousarray(om.reshape(nit, 128).T)          # [128, nit]
        in_maps.append({
            "xT": xT_sh, "wq": wq_t, "wk": wk_t, "wv": wv_t, "wo": wo_t,
            "bq": bq_t, "negmask": nm_t, "outmask": om_t, "bo_eff": bo_eff,
            "pair_bk": np.array([[(1 - h) * (E // 2)]], dtype=np.uint32),
            "pair_bv": np.array([[(1 - h) * nr]], dtype=np.uint32),
        })
    return in_maps


def kernel(x, mask, WQ_w, WQ_b, WK_w, WK_b, WV_w, WV_b, WO_w, WO_b):
    mask = np.asarray(mask).astype(bool)
    in_maps = build_in_maps(x, mask, WQ_w, WQ_b, WK_w, WK_b, WV_w, WV_b, WO_w, WO_b)
    idxs, nr = _compaction(mask)
    results = run_spmd(in_maps, nr)
    out = np.zeros((B, S, E), dtype=np.float32)
    for c in range(N_CORES):
        b, h = divmod(c, 2)
        nv = len(idxs[c])
        out[b, h * ROWS + idxs[c], :] = results[c]["out"][:nv].astype(np.float32)
    return out
